# revision 55
# baseline (speedup 1.0000x reference)
"""Bahdanau attention forward on 8 Trainium2 NeuronCores (fp8 DoubleRow).

reference:
    qh     = h_t @ W_h.T                     [B, D]
    kh     = keys @ W_k.T                    [B, N, D]
    energy = tanh(qh[:, None, :] + kh)       [B, N, D]
    scores = energy @ v                      [B, N]
    alpha  = softmax(scores, -1)             [B, N]
    context= alpha @ keys                    [B, D]
    return (context, alpha)

Sharding: data-parallel over batch B=64 across 8 cores (8 batches/core);
weights replicated. No cross-core communication.

The dominant cost is kh (2*N*D*D = 2.1 GFLOP/batch). It runs as an
e4m3 DoubleRow matmul (157 TF/s, 2x bf16): keys and 64*W_k are quantized
to TRN fp8_e4m3 on the host. The fp8 quantization noise would push alpha
past the 2e-2 gate (2.3e-2), so a first-order Taylor correction of the
scores is applied: with dW = W8/64 - W, dk = k8 - keys and c ~ E[tanh'],

    scores ~= v.T tanh(qh + kh8) - c*(k8 @ (dW.T v) + dk @ (W8.T v / 64))

Both correction terms are [N,D]@[D,1] matvecs against fp8 operands
already in SBUF (k8T for kh; dk8T shipped as e4m3(256*dk)), so they ride
the same DoubleRow path and accumulate straight into the scores PSUM:
v is shipped pre-scaled by 65536 so the scores psum, the w1 = -c*65536*dW.Tv
matvec and the u8 = -c*256*(W8.Tv/64) matvec (times the 256 inside dk8T)
all land at 65536x natural scale; Exp then applies scale=1/65536.
Simulated end-to-end error: alpha 7.8e-3, context 4.4e-3 (gate 2e-2).

Per-core device pipeline:
  - host pre-transposes keys: kT8[B,D,N] e4m3 + dk8T[B,D,N] e4m3 ride the
    sync HWDGE ring as plain DMAs (no xbar transposes at all); knat bf16
    natural layout rides SWDGE for the context matmul.
  - khT[e, n] = W8T.T @ kT8 per 128-row e-tile via DoubleRow (2 d-subtiles
    per instruction), accumulated in PSUM
  - energyT = tanh(khT/64 + qh) on ScalarE with per-partition bias qhT[:, b]
  - scores psum [64,512] rows 0/32 (nh column-paired): v-as-weights bf16
    matmuls one e-tile late, then the two fp8 correction matvecs accumulate
    into the same rows
  - softmax: Exp reads the scores PSUM rows with scale=1/65536 + accum_out
    partial sums (scores are O(1): no max-shift)
  - alphaT via K=1 matmul transpose; context[1, d] += alphaT_nt.T @ knat_nt
    with the two 512-halves in PE column groups 0/1
  - batch b's alphaT/context matmuls are emitted after batch b+1's kh so the
    PE never waits on softmax; keys prefetched 2 batches ahead; warmup
    matmuls keep the PE HAM clock at 8/8 through the initial load.
"""

import os
import numpy as np
import ml_dtypes

B, N, D = 64, 1024, 1024
NCORES = 8
B_LOC = B // NCORES
P = 128
ET = D // P
DT = D // P
NT = N // P
NH = N // 512  # 512-wide psum column halves
C_TAYLOR = 0.68
SC_SCALE = 65536.0

_compiled = None


def _emit(nc, tc, ctx, aps):
    import concourse.mybir as mybir

    f32 = mybir.dt.float32
    bf16 = mybir.dt.bfloat16
    f8 = mybir.dt.float8e4
    Tanh = mybir.ActivationFunctionType.Tanh
    Exp = mybir.ActivationFunctionType.Exp
    DR = mybir.MatmulPerfMode.DoubleRow

    knat_l, kt8_l, dk8_l, w8T, whT, wsm, vf32, ctx_out, alpha_out = aps

    consts = ctx.enter_context(tc.tile_pool(name="consts", bufs=1))
    knat_pool = ctx.enter_context(tc.tile_pool(name="knat", bufs=4))
    kT_pool = ctx.enter_context(tc.tile_pool(name="kT", bufs=3))
    dkT_pool = ctx.enter_context(tc.tile_pool(name="dkT", bufs=3))
    sm1_pool = ctx.enter_context(tc.tile_pool(name="sm1", bufs=1))
    en_pool = ctx.enter_context(tc.tile_pool(name="energy", bufs=3))
    sm_pool = ctx.enter_context(tc.tile_pool(name="sm", bufs=2))
    acc_pool = ctx.enter_context(tc.tile_pool(name="acc", bufs=2))
    sctmp_pool = ctx.enter_context(tc.tile_pool(name="sctmp", bufs=2))
    psum_kh = ctx.enter_context(tc.tile_pool(name="psum_kh", bufs=2, space="PSUM"))
    psum_sc = ctx.enter_context(tc.tile_pool(name="psum_sc", bufs=2, space="PSUM"))
    psum_misc = ctx.enter_context(tc.tile_pool(name="psum_misc", bufs=2, space="PSUM"))

    # keys loads, prefetched PF batches ahead of compute
    PF = 2
    knats: dict[int, object] = {}
    kTs: dict[int, object] = {}
    dkTs: dict[int, object] = {}

    def prefetch_kt(b):
        if b >= B_LOC:
            return
        kT = kT_pool.tile([P, DT, N], f8, tag="kT", name=f"kT{b}")
        nc.sync.dma_start(out=kT[:], in_=kt8_l[b].rearrange("(dt p) n -> p dt n", p=P))
        kTs[b] = kT
        dkT = dkT_pool.tile([P, DT, N], f8, tag="dkT", name=f"dkT{b}")
        nc.sync.dma_start(out=dkT[:], in_=dk8_l[b].rearrange("(dt p) n -> p dt n", p=P))
        dkTs[b] = dkT

    def prefetch_knat(b):
        # knat(b) is first read by tail_ctx(b) during batch b+1. The tile
        # scheduler hoists dependency-free DMAs to the very front, which
        # starves the critical kT8(b0) DMA (startup is HBM-bandwidth-bound),
        # so gate each knat(b) DMA on kT8(b)'s arrival with a dummy 1-elem
        # copy into the tile (WAW forces the DMA to wait).
        if b >= B_LOC or b in knats:
            return
        knat = knat_pool.tile([P, NT, D], bf16, tag="knat", name=f"knat{b}")
        # 1-elem gate DMA on the gpsimd queue (only deadline-free output DMAs
        # live there; a vector-op gate blocked the softmax chain head-of-line)
        nc.gpsimd.dma_start(out=knat[0:1, 0, 0:1], in_=kTs[b][0:1, 0, 0:1])
        nc.gpsimd.dma_start(
            out=knat[:], in_=knat_l[b].rearrange("(nt p) d -> p nt d", p=P)
        )
        knats[b] = knat

    def tail_pat(b, ex_bf):
        """exT transposes for batch b: the UNNORMALIZED exp is transposed
        (available right after Exp, before the sum/reciprocal finish) and the
        1/sum scales the context on the way out of PSUM instead. bf16
        operands: fp32 matmuls run in multi-pass LOW_HIGH mode at ~2.4x."""
        pat = psum_misc.tile([P, NT], f32, tag="misc", name=f"pat{b}")
        for nt in range(NT):
            nc.tensor.matmul(
                pat[:, nt : nt + 1],
                ex_bf[0:1, nt * P : (nt + 1) * P],
                ones_bf[:],
                start=True,
                stop=True,
            )
        exT_sb = sm_pool.tile([P, NT], bf16, tag="alphaT", name=f"exT{b}")
        nc.vector.tensor_copy(out=exT_sb[:], in_=pat[:])
        return exT_sb

    def tail_ctx(b, exT_sb, rcp):
        knat = knats.pop(b)
        cxp = psum_misc.tile([64, 512], f32, tag="misc", name=f"cx{b}")
        for nt in range(NT):
            for nh in range(NH):
                nc.tensor.matmul(
                    cxp[32 * nh : 32 * nh + 1, :],
                    exT_sb[:, nt : nt + 1],
                    knat[:, nt, nh * 512 : (nh + 1) * 512],
                    start=(nt == 0),
                    stop=(nt == NT - 1),
                    tile_position=(0, 32 * nh),
                )
        ctx_sb = sm_pool.tile([64, 512], f32, tag="ctx_sb", name=f"ctx_sb{b}")
        for nh in range(NH):
            nc.vector.tensor_scalar_mul(
                ctx_sb[32 * nh : 32 * nh + 1, :],
                cxp[32 * nh : 32 * nh + 1, :],
                rcp[:],
            )
            nc.gpsimd.dma_start(
                out=ctx_out[b : b + 1, nh * 512 : (nh + 1) * 512],
                in_=ctx_sb[32 * nh : 32 * nh + 1, :],
            )

    # consts: w8T (with the correction vectors packed as cols 1024/1025,
    # padded to 1040 so the DoubleRow dt-stride stays % 16B == 0) leads the
    # sync ring ahead of kT8(b0); wsm (h_t.T | 65536*v, tiny) then whT (2KB
    # aligned rows -- a combined pack had 2066B rows and crawled at 84GB/s)
    # ride the scalar queue in parallel.
    W8C = D + 16
    w8_sb = consts.tile([P, DT, W8C], f8)
    nc.sync.dma_start(out=w8_sb[:], in_=w8T.rearrange("(dt p) c -> p dt c", p=P))
    wsm_sb = consts.tile([P, DT, 24], bf16)
    nc.scalar.dma_start(out=wsm_sb[:], in_=wsm.rearrange("(dt p) c -> p dt c", p=P))
    whT_sb = consts.tile([P, DT, D], bf16)
    nc.scalar.dma_start(out=whT_sb[:], in_=whT.rearrange("(dt p) c -> p dt c", p=P))
    vf_sb = consts.tile([P, DT, 1], f32)
    nc.scalar.dma_start(out=vf_sb[:], in_=vf32.rearrange("(dt p) c -> p dt c", p=P))
    w8T_sb = w8_sb[:, :, 0:D]
    w1_sb = w8_sb[:, :, D : D + 1]
    u8_sb = w8_sb[:, :, D + 1 : D + 2]
    htT_sb = wsm_sb[:, :, 0:B_LOC]
    ident8 = wsm_sb[0:B_LOC, 0, 16 : 16 + B_LOC]
    ones_bf = consts.tile([1, 1], bf16)
    nc.gpsimd.memset(ones_bf[:], 1.0)
    ones_col = consts.tile([P, 1], bf16)
    nc.gpsimd.memset(ones_col[:], 1.0)
    warm_src = consts.tile([P, 512], bf16)
    nc.gpsimd.memset(warm_src[:], 0.0)

    for b in range(min(PF, B_LOC)):
        prefetch_kt(b)
    for b in range(min(PF, B_LOC)):
        prefetch_knat(b)

    # HAM warmup + fill the PE while the consts + first keys batch load
    wp = psum_misc.tile([P, 512], f32, tag="misc", name="warmup")
    for w in range(20):
        nc.tensor.matmul(wp[:], warm_src[:, :P], warm_src[:], start=True, stop=True)

    # qhT[e-tile, b] = (h_t @ W_h.T).T. Computed flipped (stationary h_t.T,
    # M=8) so each dt needs one cheap 8-col weight load instead of a 128-col
    # one, then transposed back per e-tile with is_transpose matmuls against
    # an 8x8 identity: ~4.5us of PE vs 10.8us for the e-tile-major version.
    qhT_sb = consts.tile([P, ET, B_LOC], bf16)
    qh_sb = consts.tile([B_LOC, N], bf16)

    def emit_qh():
        # emitted after kh(b0, et0) so the 2MB whT DMA never blocks kh(b0)
        for nh in range(NH):
            qp = psum_misc.tile([B_LOC, 512], f32, tag="misc", name=f"qp{nh}")
            for dt in range(DT):
                nc.tensor.matmul(
                    qp[:],
                    htT_sb[:, dt, :],
                    whT_sb[:, dt, nh * 512 : (nh + 1) * 512],
                    start=(dt == 0),
                    stop=(dt == DT - 1),
                )
            nc.vector.tensor_copy(out=qh_sb[:, nh * 512 : (nh + 1) * 512], in_=qp[:])
        for et in range(ET):
            tp = psum_misc.tile([P, B_LOC], bf16, tag="misc", name=f"qhT{et}")
            nc.tensor.matmul(
                tp[:],
                qh_sb[:, et * P : (et + 1) * P],
                ident8,
                is_transpose=True,
            )
            nc.vector.tensor_copy(out=qhT_sb[:, et, :], in_=tp[:])

    pending = None
    pending_alphaT = None

    for b in range(B_LOC):
        kT = kTs.pop(b)
        dkT = dkTs.pop(b)

        # scores accumulator [64, 512]: nh half nh lives at row 32*nh
        sc = psum_sc.tile([64, 512], f32, tag="sc", name=f"sc{b}")

        def corr_mms(vec, rhs_t, first):
            # fp8 Taylor-correction matvec accumulating into the scores rows.
            # Plain fp8 (no DoubleRow): column pairing and DoubleRow are
            # mutually exclusive (XBUS budget); M=1 runs at column rate anyway.
            for dt in range(DT):
                for nh in range(NH):
                    nc.tensor.matmul(
                        sc[32 * nh : 32 * nh + 1, :],
                        vec[:, dt, :],
                        rhs_t[:, dt, nh * 512 : (nh + 1) * 512],
                        start=(first and dt == 0),
                        stop=False,
                        tile_position=(0, 32 * nh),
                    )

        # the e-contraction of scores (v.T energy) rides the DVE: per e-tile
        # acc += en * v_et (per-partition scalar, bf16), then one ones^T @ acc
        # matmul per nh folds the 128 partitions into the scores psum. This
        # takes 8 column-paired PE slots per batch off the critical engine.
        acc = None
        for et in range(ET):
            pk = psum_kh.tile([P, N], f32, tag="kh")
            for dtp in range(DT // 2):
                lhsT = w8T_sb[:, 2 * dtp : 2 * dtp + 2, et * P : (et + 1) * P]
                for nh in range(NH):
                    nc.tensor.matmul(
                        pk[:, nh * 512 : (nh + 1) * 512],
                        lhsT,
                        kT[:, 2 * dtp : 2 * dtp + 2, nh * 512 : (nh + 1) * 512],
                        start=(dtp == 0),
                        stop=(dtp == DT // 2 - 1),
                        perf_mode=DR,
                    )
            if b == 0 and et == 0:
                emit_qh()
            if pending is not None:
                if et == 2:
                    pending_alphaT = tail_pat(pending[0], pending[1])
                elif et == 5:
                    tail_ctx(pending[0], pending_alphaT, pending[2])
            en = en_pool.tile([P, N], bf16, tag="en")
            nc.scalar.activation(
                out=en[:],
                in_=pk[:],
                func=Tanh,
                bias=qhT_sb[:, et, b : b + 1],
                scale=1.0 / 64.0,
            )
            v_ap = vf_sb[:, et, :]
            if acc is None:
                acc = acc_pool.tile([P, N], bf16, tag="acc", name=f"acc{b}_0")
                nc.vector.tensor_scalar_mul(acc[:], en[:], v_ap)
            else:
                tmp = sctmp_pool.tile([P, N], bf16, tag="sctmp")
                nc.vector.tensor_scalar_mul(tmp[:], en[:], v_ap)
                acc2 = acc_pool.tile([P, N], bf16, tag="acc", name=f"acc{b}_{et}")
                nc.vector.tensor_add(acc2[:], acc[:], tmp[:])
                acc = acc2
        corr_mms(w1_sb, kT, first=True)
        corr_mms(u8_sb, dkT, first=False)
        for nh in range(NH):
            nc.tensor.matmul(
                sc[32 * nh : 32 * nh + 1, :],
                ones_col[:],
                acc[:, nh * 512 : (nh + 1) * 512],
                start=False,
                stop=True,
                tile_position=(0, 32 * nh),
            )

        # softmax over [1, N]: exp straight from the scores PSUM rows (ScE
        # reads PSUM fastest); scores are O(1) so fp32 exp needs no max-shift
        ex = sm1_pool.tile([1, N], f32, tag="ex")
        ssums = sm_pool.tile([1, 2], f32, tag="ssums")
        for nh in range(NH):
            nc.scalar.activation(
                out=ex[:, nh * 512 : (nh + 1) * 512],
                in_=sc[32 * nh : 32 * nh + 1, :],
                func=Exp,
                bias=0.0,
                scale=1.0 / SC_SCALE,
                accum_out=ssums[:, nh : nh + 1],
            )
        # bf16 copy of the raw exp feeds the exT transposes immediately
        ex_bf = sm_pool.tile([1, N], bf16, tag="ex_bf", name=f"ex_bf{b}")
        nc.vector.tensor_copy(out=ex_bf[:], in_=ex[:])
        ssum = sm_pool.tile([1, 1], f32, tag="ssum")
        nc.vector.tensor_add(ssum[:], ssums[:, 0:1], ssums[:, 1:2])
        rcp = sm_pool.tile([1, 1], f32, tag="rcp", name=f"rcp{b}")
        nc.vector.reciprocal(rcp[:], ssum[:])
        alpha_sb = sm_pool.tile([1, N], f32, tag="alpha_sb", name=f"alpha_sb{b}")
        nc.vector.tensor_scalar_mul(alpha_sb[:], ex[:], rcp[:])
        nc.gpsimd.dma_start(out=alpha_out[b : b + 1, :], in_=alpha_sb[:])

        pending = (b, ex_bf, rcp)
        prefetch_kt(b + PF)
        prefetch_knat(b + PF)

    tail_ctx(pending[0], tail_pat(pending[0], pending[1]), pending[2])


def _build():
    from contextlib import ExitStack

    import concourse.mybir as mybir
    import concourse.tile as tile
    from concourse import bacc

    f32 = mybir.dt.float32
    bf16 = mybir.dt.bfloat16
    f8 = mybir.dt.float8e4

    nc = bacc.Bacc("TRN2", target_bir_lowering=False, debug=False, num_devices=NCORES)
    knat_l = nc.dram_tensor("knat_l", [B_LOC, N, D], bf16, kind="ExternalInput")
    kt8_l = nc.dram_tensor("kt8_l", [B_LOC, D, N], f8, kind="ExternalInput")
    dk8_l = nc.dram_tensor("dk8_l", [B_LOC, D, N], f8, kind="ExternalInput")
    # packed consts: fp8 w8T [d, 0:D] = 64*W_k.T quantized, [d, D]=w1_8,
    # [d, D+1]=u8_8, padded to D+16 cols so the DoubleRow dt-stride is % 16;
    # bf16 whT = W_h.T; wsm [d, 0:8]=h_t.T, [d, 8]=65536*v
    w8T = nc.dram_tensor("w8T", [D, D + 16], f8, kind="ExternalInput")
    whT = nc.dram_tensor("whT", [D, D], bf16, kind="ExternalInput")
    # wsm: [d, 0:8]=h_t.T, [d, 8]=65536*v, [0:8, 16:24]=I8 (transpose perm)
    wsm = nc.dram_tensor("wsm", [D, 24], bf16, kind="ExternalInput")
    vf32 = nc.dram_tensor("vf32", [D, 1], f32, kind="ExternalInput")
    ctx_out = nc.dram_tensor("ctx_out", [B_LOC, D], f32, kind="ExternalOutput")
    alpha_out = nc.dram_tensor("alpha_out", [B_LOC, N], f32, kind="ExternalOutput")

    aps = (
        knat_l.ap(),
        kt8_l.ap(),
        dk8_l.ap(),
        w8T.ap(),
        whT.ap(),
        wsm.ap(),
        vf32.ap(),
        ctx_out.ap(),
        alpha_out.ap(),
    )
    with tile.TileContext(nc) as tc:
        with ExitStack() as ctx:
            _emit(nc, tc, ctx, aps)
    nc.compile()
    return nc


def _get_compiled():
    global _compiled
    if _compiled is None:
        _compiled = _build()
    return _compiled


def _install_prof_shim():
    """Shim antenv.axon_hooks so run_bass_kernel_spmd(trace=True) can
    NTFF-profile under axon; neuter the bucket artifact upload."""
    import sys
    import types

    if "antenv.axon_hooks" not in sys.modules:
        import antenv

        mod = types.ModuleType("antenv.axon_hooks")
        mod._hook = None
        mod.set_axon_ntff_profile_hook = lambda h: setattr(mod, "_hook", h)
        mod.get_axon_ntff_profile_hook = lambda: mod._hook
        sys.modules["antenv.axon_hooks"] = mod
        antenv.axon_hooks = mod
        try:
            from trn_agent_boot.trn_boot import _ntff_profile_via_ctypes

            mod._hook = _ntff_profile_via_ctypes("/opt/axon/libaxon_pjrt.so")
        except Exception:
            pass

    from concourse import bass_utils

    bass_utils.upload_artifacts = lambda tmpdir: f"local://{tmpdir}"


def host_prep(h_t, keys, W_h, W_k, v):
    bf = ml_dtypes.bfloat16
    e4 = ml_dtypes.float8_e4m3
    f32 = np.float32
    h_t = np.asarray(h_t, dtype=f32)
    keys = np.asarray(keys, dtype=f32)
    W_h = np.asarray(W_h, dtype=f32)
    W_k = np.asarray(W_k, dtype=f32)
    v = np.asarray(v, dtype=f32)

    def q8(x):
        return np.clip(x, -240.0, 240.0).astype(e4)

    # keys in three forms: bf16 natural, e4m3 transposed, e4m3 residual x256
    knat = keys.astype(bf)
    keys_T = np.ascontiguousarray(keys.transpose(0, 2, 1))  # [B, D, N]
    kt8 = q8(keys_T)
    dk8 = q8(256.0 * (kt8.astype(f32) - keys_T))

    # weights: W8 = e4m3(64*W_k); correction vectors (host fp32)
    W8s = q8(64.0 * W_k)
    W8f = W8s.astype(f32)
    w1 = (W8f / 64.0 - W_k).T @ v
    u8 = (W8f.T @ v) / 64.0
    w1_8 = q8(-C_TAYLOR * SC_SCALE * w1).reshape(D, 1)
    u8_8 = q8(-C_TAYLOR * 256.0 * u8).reshape(D, 1)
    pad8 = np.zeros((D, 14), dtype=e4)
    w8T_arr = np.concatenate([np.ascontiguousarray(W8s.T), w1_8, u8_8, pad8], axis=1)

    whT_arr = np.ascontiguousarray(W_h.T).astype(bf)
    v_s = (SC_SCALE * v).astype(bf).reshape(D, 1)
    v_f = (SC_SCALE * v).astype(f32).reshape(D, 1)
    tail_cols = np.zeros((D, 24 - B_LOC - 1), dtype=bf)
    tail_cols[0:B_LOC, 7 : 7 + B_LOC] = np.eye(B_LOC, dtype=bf)

    in_maps = []
    for c in range(NCORES):
        sl = slice(c * B_LOC, (c + 1) * B_LOC)
        htT = np.ascontiguousarray(h_t[sl].T).astype(bf)
        wsm_arr = np.concatenate([htT, v_s, tail_cols], axis=1)
        in_maps.append(
            {
                "knat_l": knat[sl],
                "kt8_l": kt8[sl],
                "dk8_l": dk8[sl],
                "w8T": w8T_arr,
                "whT": whT_arr,
                "wsm": wsm_arr,
                "vf32": v_f,
            }
        )
    return in_maps


def kernel(h_t, keys, W_h, W_k, v):
    from concourse import bass_utils

    in_maps = host_prep(h_t, keys, W_h, W_k, v)
    nc = _get_compiled()

    trace = os.environ.get("BAHDANAU_TRACE", "0") == "1"
    if trace:
        _install_prof_shim()
    res = bass_utils.run_bass_kernel_spmd(
        nc, in_maps, core_ids=list(range(NCORES)), trace=trace
    )
    if trace:
        kernel.last_exec_time_ns = res.exec_time_ns
        kernel.last_results = res

    context = np.concatenate([res.results[c]["ctx_out"] for c in range(NCORES)], axis=0)
    alpha = np.concatenate([res.results[c]["alpha_out"] for c in range(NCORES)], axis=0)
    return (context, alpha)


# revision 62
# speedup vs baseline: 1.2148x; 1.2148x over previous
"""Bahdanau attention forward on 8 Trainium2 NeuronCores (fp8 DoubleRow).

reference:
    qh     = h_t @ W_h.T                     [B, D]
    kh     = keys @ W_k.T                    [B, N, D]
    energy = tanh(qh[:, None, :] + kh)       [B, N, D]
    scores = energy @ v                      [B, N]
    alpha  = softmax(scores, -1)             [B, N]
    context= alpha @ keys                    [B, D]
    return (context, alpha)

Sharding: data-parallel over batch B=64 across 8 cores (8 batches/core);
weights replicated. No cross-core communication.

The dominant cost is kh (2*N*D*D = 2.1 GFLOP/batch). It runs as an
e4m3 DoubleRow matmul (157 TF/s, 2x bf16): keys and 64*W_k are quantized
to TRN fp8_e4m3 on the host. The fp8 quantization noise would push alpha
past the 2e-2 gate (2.3e-2), so a first-order Taylor correction of the
scores is applied: with dW = W8/64 - W, dk = k8 - keys and c ~ E[tanh'],

    scores ~= v.T tanh(qh + kh8) - c*(k8 @ (dW.T v) + dk @ (W8.T v / 64))

Both correction terms are [N,D]@[D,1] matvecs against fp8 operands
already in SBUF (k8T for kh; dk8T shipped as e4m3(256*dk)), so they ride
the same DoubleRow path and accumulate straight into the scores PSUM:
v is shipped pre-scaled by 65536 so the scores psum, the w1 = -c*65536*dW.Tv
matvec and the u8 = -c*256*(W8.Tv/64) matvec (times the 256 inside dk8T)
all land at 65536x natural scale; Exp then applies scale=1/65536.
Simulated end-to-end error: alpha 7.8e-3, context 4.4e-3 (gate 2e-2).

Per-core device pipeline:
  - host pre-transposes keys: kT8[B,D,N] e4m3 + dk8T[B,D,N] e4m3 ride the
    sync HWDGE ring as plain DMAs (no xbar transposes at all); knat bf16
    natural layout rides SWDGE for the context matmul.
  - khT[e, n] = W8T.T @ kT8 per 128-row e-tile via DoubleRow (2 d-subtiles
    per instruction), accumulated in PSUM
  - energyT = tanh(khT/64 + qh) on ScalarE with per-partition bias qhT[:, b]
  - scores psum [64,512] rows 0/32 (nh column-paired): v-as-weights bf16
    matmuls one e-tile late, then the two fp8 correction matvecs accumulate
    into the same rows
  - softmax: Exp reads the scores PSUM rows with scale=1/65536 + accum_out
    partial sums (scores are O(1): no max-shift)
  - alphaT via K=1 matmul transpose; context[1, d] += alphaT_nt.T @ knat_nt
    with the two 512-halves in PE column groups 0/1
  - batch b's alphaT/context matmuls are emitted after batch b+1's kh so the
    PE never waits on softmax; keys prefetched 2 batches ahead; warmup
    matmuls keep the PE HAM clock at 8/8 through the initial load.
"""

import os
import numpy as np
import ml_dtypes

B, N, D = 64, 1024, 1024
NCORES = 8
B_LOC = B // NCORES
P = 128
ET = D // P
DT = D // P
NT = N // P
NH = N // 512  # 512-wide psum column halves
C_TAYLOR = 0.68
SC_SCALE = 65536.0

_compiled = None


def _emit(nc, tc, ctx, aps):
    import concourse.mybir as mybir

    f32 = mybir.dt.float32
    bf16 = mybir.dt.bfloat16
    f8 = mybir.dt.float8e4
    Tanh = mybir.ActivationFunctionType.Tanh
    Exp = mybir.ActivationFunctionType.Exp
    DR = mybir.MatmulPerfMode.DoubleRow

    knat_l, kt8_l, dk8_l, w8T, whT, wsm, vf32, ctx_out, alpha_out = aps

    consts = ctx.enter_context(tc.tile_pool(name="consts", bufs=1))
    knat_pool = ctx.enter_context(tc.tile_pool(name="knat", bufs=4))
    kT_pool = ctx.enter_context(tc.tile_pool(name="kT", bufs=3))
    dkT_pool = ctx.enter_context(tc.tile_pool(name="dkT", bufs=3))
    sm1_pool = ctx.enter_context(tc.tile_pool(name="sm1", bufs=1))
    en_pool = ctx.enter_context(tc.tile_pool(name="energy", bufs=3))
    sm_pool = ctx.enter_context(tc.tile_pool(name="sm", bufs=2))
    acc_pool = ctx.enter_context(tc.tile_pool(name="acc", bufs=2))
    sctmp_pool = ctx.enter_context(tc.tile_pool(name="sctmp", bufs=2))
    psum_kh = ctx.enter_context(tc.tile_pool(name="psum_kh", bufs=2, space="PSUM"))
    psum_sc = ctx.enter_context(tc.tile_pool(name="psum_sc", bufs=2, space="PSUM"))
    psum_misc = ctx.enter_context(tc.tile_pool(name="psum_misc", bufs=2, space="PSUM"))

    # keys loads, prefetched PF batches ahead of compute
    PF = 2
    knats: dict[int, object] = {}
    kTs: dict[int, object] = {}
    dkTs: dict[int, object] = {}

    def prefetch_kt(b):
        if b >= B_LOC:
            return
        kT = kT_pool.tile([P, DT, N], f8, tag="kT", name=f"kT{b}")
        nc.sync.dma_start(out=kT[:], in_=kt8_l[b].rearrange("(dt p) n -> p dt n", p=P))
        kTs[b] = kT
        dkT = dkT_pool.tile([P, DT, N], f8, tag="dkT", name=f"dkT{b}")
        nc.sync.dma_start(out=dkT[:], in_=dk8_l[b].rearrange("(dt p) n -> p dt n", p=P))
        dkTs[b] = dkT

    def prefetch_knat(b):
        # knat(b) is first read by tail_ctx(b) during batch b+1. The tile
        # scheduler hoists dependency-free DMAs to the very front, which
        # starves the critical kT8(b0) DMA (startup is HBM-bandwidth-bound),
        # so gate each knat(b) DMA on kT8(b)'s arrival with a dummy 1-elem
        # copy into the tile (WAW forces the DMA to wait).
        if b >= B_LOC or b in knats:
            return
        knat = knat_pool.tile([P, NT, D], bf16, tag="knat", name=f"knat{b}")
        # 1-elem gate DMA on the gpsimd queue (only deadline-free output DMAs
        # live there; a vector-op gate blocked the softmax chain head-of-line)
        nc.gpsimd.dma_start(out=knat[0:1, 0, 0:1], in_=kTs[b][0:1, 0, 0:1])
        nc.gpsimd.dma_start(
            out=knat[:], in_=knat_l[b].rearrange("(nt p) d -> p nt d", p=P)
        )
        knats[b] = knat

    def tail_pat(b, alpha_sb):
        """alphaT transposes for batch b (bf16 operands: fp32 matmuls run in
        multi-pass LOW_HIGH mode at ~2.4x the cost)."""
        pat = psum_misc.tile([P, NT], f32, tag="misc", name=f"pat{b}")
        for nt in range(NT):
            nc.tensor.matmul(
                pat[:, nt : nt + 1],
                alpha_sb[0:1, nt * P : (nt + 1) * P],
                ones_bf[:],
                start=True,
                stop=True,
            )
        alphaT_sb = sm_pool.tile([P, NT], bf16, tag="alphaT", name=f"alphaT{b}")
        nc.vector.tensor_copy(out=alphaT_sb[:], in_=pat[:])
        return alphaT_sb

    def tail_ctx(b, alphaT_sb):
        knat = knats.pop(b)
        cxp = psum_misc.tile([64, 512], f32, tag="misc", name=f"cx{b}")
        for nt in range(NT):
            for nh in range(NH):
                nc.tensor.matmul(
                    cxp[32 * nh : 32 * nh + 1, :],
                    alphaT_sb[:, nt : nt + 1],
                    knat[:, nt, nh * 512 : (nh + 1) * 512],
                    start=(nt == 0),
                    stop=(nt == NT - 1),
                    tile_position=(0, 32 * nh),
                )
        ctx_sb = sm_pool.tile([64, 512], f32, tag="ctx_sb", name=f"ctx_sb{b}")
        for nh in range(NH):
            nc.vector.tensor_copy(
                out=ctx_sb[32 * nh : 32 * nh + 1, :],
                in_=cxp[32 * nh : 32 * nh + 1, :],
            )
            nc.gpsimd.dma_start(
                out=ctx_out[b : b + 1, nh * 512 : (nh + 1) * 512],
                in_=ctx_sb[32 * nh : 32 * nh + 1, :],
            )

    # consts: w8T (with the correction vectors packed as cols 1024/1025,
    # padded to 1040 so the DoubleRow dt-stride stays % 16B == 0) leads the
    # sync ring ahead of kT8(b0); wsm (h_t.T | 65536*v, tiny) then whT (2KB
    # aligned rows -- a combined pack had 2066B rows and crawled at 84GB/s)
    # ride the scalar queue in parallel.
    W8C = D + 16
    w8_sb = consts.tile([P, DT, W8C], f8)
    nc.sync.dma_start(out=w8_sb[:], in_=w8T.rearrange("(dt p) c -> p dt c", p=P))
    wsm_sb = consts.tile([P, DT, 24], bf16)
    nc.scalar.dma_start(out=wsm_sb[:], in_=wsm.rearrange("(dt p) c -> p dt c", p=P))
    # whT split into per-dt chunk DMAs: qh's per-dt accumulation can start on
    # chunk 0 instead of stalling ~6us on the full 2MB transfer
    whT_sb = consts.tile([P, DT, D], bf16)
    for dt in range(DT):
        nc.scalar.dma_start(out=whT_sb[:, dt, :], in_=whT[dt * P : (dt + 1) * P, :])
    vf_sb = consts.tile([P, DT, 1], f32)
    nc.scalar.dma_start(out=vf_sb[:], in_=vf32.rearrange("(dt p) c -> p dt c", p=P))
    w8T_sb = w8_sb[:, :, 0:D]
    w1_sb = w8_sb[:, :, D : D + 1]
    u8_sb = w8_sb[:, :, D + 1 : D + 2]
    htT_sb = wsm_sb[:, :, 0:B_LOC]
    ident8 = wsm_sb[0:B_LOC, 0, 16 : 16 + B_LOC]
    ones_bf = consts.tile([1, 1], bf16)
    nc.gpsimd.memset(ones_bf[:], 1.0)
    ones_col = consts.tile([P, 1], bf16)
    nc.gpsimd.memset(ones_col[:], 1.0)
    warm_src = consts.tile([P, 512], bf16)
    nc.gpsimd.memset(warm_src[:], 0.0)

    for b in range(min(PF, B_LOC)):
        prefetch_kt(b)
    for b in range(min(PF, B_LOC)):
        prefetch_knat(b)

    # HAM warmup + fill the PE while the consts + first keys batch load
    wp = psum_misc.tile([P, 512], f32, tag="misc", name="warmup")
    for w in range(20):
        nc.tensor.matmul(wp[:], warm_src[:, :P], warm_src[:], start=True, stop=True)

    # qhT[e-tile, b] = (h_t @ W_h.T).T. Computed flipped (stationary h_t.T,
    # M=8) so each dt needs one cheap 8-col weight load instead of a 128-col
    # one, then transposed back per e-tile with is_transpose matmuls against
    # an 8x8 identity: ~4.5us of PE vs 10.8us for the e-tile-major version.
    qhT_sb = consts.tile([P, ET, B_LOC], bf16)
    qh_sb = consts.tile([B_LOC, N], bf16)

    def emit_qh():
        for nh in range(NH):
            qp = psum_misc.tile([B_LOC, 512], f32, tag="misc", name=f"qp{nh}")
            for dt in range(DT):
                nc.tensor.matmul(
                    qp[:],
                    htT_sb[:, dt, :],
                    whT_sb[:, dt, nh * 512 : (nh + 1) * 512],
                    start=(dt == 0),
                    stop=(dt == DT - 1),
                )
            nc.vector.tensor_copy(out=qh_sb[:, nh * 512 : (nh + 1) * 512], in_=qp[:])
        for et in range(ET):
            tp = psum_misc.tile([P, B_LOC], bf16, tag="misc", name=f"qhT{et}")
            nc.tensor.matmul(
                tp[:],
                qh_sb[:, et * P : (et + 1) * P],
                ident8,
                is_transpose=True,
            )
            nc.vector.tensor_copy(out=qhT_sb[:, et, :], in_=tp[:])

    emit_qh()
    pending = None
    pending_alphaT = None

    for b in range(B_LOC):
        kT = kTs.pop(b)
        dkT = dkTs.pop(b)

        # scores accumulator [64, 512]: nh half nh lives at row 32*nh
        sc = psum_sc.tile([64, 512], f32, tag="sc", name=f"sc{b}")

        def corr_mms(vec, rhs_t, first):
            # fp8 Taylor-correction matvec accumulating into the scores rows.
            # Plain fp8 (no DoubleRow): column pairing and DoubleRow are
            # mutually exclusive (XBUS budget); M=1 runs at column rate anyway.
            for dt in range(DT):
                for nh in range(NH):
                    nc.tensor.matmul(
                        sc[32 * nh : 32 * nh + 1, :],
                        vec[:, dt, :],
                        rhs_t[:, dt, nh * 512 : (nh + 1) * 512],
                        start=(first and dt == 0),
                        stop=False,
                        tile_position=(0, 32 * nh),
                    )

        # the e-contraction of scores (v.T energy) rides the DVE: per e-tile
        # acc += en * v_et (per-partition scalar, bf16), then one ones^T @ acc
        # matmul per nh folds the 128 partitions into the scores psum. This
        # takes 8 column-paired PE slots per batch off the critical engine.
        acc = None
        for et in range(ET):
            pk = psum_kh.tile([P, N], f32, tag="kh")
            for dtp in range(DT // 2):
                lhsT = w8T_sb[:, 2 * dtp : 2 * dtp + 2, et * P : (et + 1) * P]
                for nh in range(NH):
                    nc.tensor.matmul(
                        pk[:, nh * 512 : (nh + 1) * 512],
                        lhsT,
                        kT[:, 2 * dtp : 2 * dtp + 2, nh * 512 : (nh + 1) * 512],
                        start=(dtp == 0),
                        stop=(dtp == DT // 2 - 1),
                        perf_mode=DR,
                    )
            if pending is not None:
                if et == 2:
                    pending_alphaT = tail_pat(pending[0], pending[1])
                elif et == 5:
                    tail_ctx(pending[0], pending_alphaT)
            en = en_pool.tile([P, N], bf16, tag="en")
            nc.scalar.activation(
                out=en[:],
                in_=pk[:],
                func=Tanh,
                bias=qhT_sb[:, et, b : b + 1],
                scale=1.0 / 64.0,
            )
            v_ap = vf_sb[:, et, :]
            if acc is None:
                acc = acc_pool.tile([P, N], bf16, tag="acc", name=f"acc{b}_0")
                nc.vector.tensor_scalar_mul(acc[:], en[:], v_ap)
            else:
                tmp = sctmp_pool.tile([P, N], bf16, tag="sctmp")
                nc.vector.tensor_scalar_mul(tmp[:], en[:], v_ap)
                acc2 = acc_pool.tile([P, N], bf16, tag="acc", name=f"acc{b}_{et}")
                nc.vector.tensor_add(acc2[:], acc[:], tmp[:])
                acc = acc2
        corr_mms(w1_sb, kT, first=True)
        corr_mms(u8_sb, dkT, first=False)
        for nh in range(NH):
            nc.tensor.matmul(
                sc[32 * nh : 32 * nh + 1, :],
                ones_col[:],
                acc[:, nh * 512 : (nh + 1) * 512],
                start=False,
                stop=True,
                tile_position=(0, 32 * nh),
            )

        # softmax over [1, N]: exp straight from the scores PSUM rows (ScE
        # reads PSUM fastest); scores are O(1) so fp32 exp needs no max-shift
        ex = sm1_pool.tile([1, N], f32, tag="ex")
        ssums = sm_pool.tile([1, 2], f32, tag="ssums")
        for nh in range(NH):
            nc.scalar.activation(
                out=ex[:, nh * 512 : (nh + 1) * 512],
                in_=sc[32 * nh : 32 * nh + 1, :],
                func=Exp,
                bias=0.0,
                scale=1.0 / SC_SCALE,
                accum_out=ssums[:, nh : nh + 1],
            )
        ssum = sm_pool.tile([1, 1], f32, tag="ssum")
        nc.vector.tensor_add(ssum[:], ssums[:, 0:1], ssums[:, 1:2])
        rcp = sm_pool.tile([1, 1], f32, tag="rcp", name=f"rcp{b}")
        nc.vector.reciprocal(rcp[:], ssum[:])
        alpha_sb = sm_pool.tile([1, N], f32, tag="alpha_sb", name=f"alpha_sb{b}")
        nc.vector.tensor_scalar_mul(alpha_sb[:], ex[:], rcp[:])
        nc.gpsimd.dma_start(out=alpha_out[b : b + 1, :], in_=alpha_sb[:])
        # bf16 copy feeds the alphaT transposes (fp32 matmul is multi-pass)
        alpha_bf = sm_pool.tile([1, N], bf16, tag="alpha_bf", name=f"alpha_bf{b}")
        nc.vector.tensor_scalar_mul(alpha_bf[:], ex[:], rcp[:])

        pending = (b, alpha_bf)
        prefetch_kt(b + PF)
        prefetch_knat(b + PF)

    tail_ctx(pending[0], tail_pat(*pending))


def _build():
    from contextlib import ExitStack

    import concourse.mybir as mybir
    import concourse.tile as tile
    from concourse import bacc

    f32 = mybir.dt.float32
    bf16 = mybir.dt.bfloat16
    f8 = mybir.dt.float8e4

    nc = bacc.Bacc("TRN2", target_bir_lowering=False, debug=False, num_devices=NCORES)
    knat_l = nc.dram_tensor("knat_l", [B_LOC, N, D], bf16, kind="ExternalInput")
    kt8_l = nc.dram_tensor("kt8_l", [B_LOC, D, N], f8, kind="ExternalInput")
    dk8_l = nc.dram_tensor("dk8_l", [B_LOC, D, N], f8, kind="ExternalInput")
    # packed consts: fp8 w8T [d, 0:D] = 64*W_k.T quantized, [d, D]=w1_8,
    # [d, D+1]=u8_8, padded to D+16 cols so the DoubleRow dt-stride is % 16;
    # bf16 whT = W_h.T; wsm [d, 0:8]=h_t.T, [d, 8]=65536*v
    w8T = nc.dram_tensor("w8T", [D, D + 16], f8, kind="ExternalInput")
    whT = nc.dram_tensor("whT", [D, D], bf16, kind="ExternalInput")
    # wsm: [d, 0:8]=h_t.T, [d, 8]=65536*v, [0:8, 16:24]=I8 (transpose perm)
    wsm = nc.dram_tensor("wsm", [D, 24], bf16, kind="ExternalInput")
    vf32 = nc.dram_tensor("vf32", [D, 1], f32, kind="ExternalInput")
    ctx_out = nc.dram_tensor("ctx_out", [B_LOC, D], f32, kind="ExternalOutput")
    alpha_out = nc.dram_tensor("alpha_out", [B_LOC, N], f32, kind="ExternalOutput")

    aps = (
        knat_l.ap(),
        kt8_l.ap(),
        dk8_l.ap(),
        w8T.ap(),
        whT.ap(),
        wsm.ap(),
        vf32.ap(),
        ctx_out.ap(),
        alpha_out.ap(),
    )
    with tile.TileContext(nc) as tc:
        with ExitStack() as ctx:
            _emit(nc, tc, ctx, aps)
    nc.compile()
    return nc


def _get_compiled():
    global _compiled
    if _compiled is None:
        _compiled = _build()
    return _compiled


def _install_prof_shim():
    """Shim antenv.axon_hooks so run_bass_kernel_spmd(trace=True) can
    NTFF-profile under axon; neuter the bucket artifact upload."""
    import sys
    import types

    if "antenv.axon_hooks" not in sys.modules:
        import antenv

        mod = types.ModuleType("antenv.axon_hooks")
        mod._hook = None
        mod.set_axon_ntff_profile_hook = lambda h: setattr(mod, "_hook", h)
        mod.get_axon_ntff_profile_hook = lambda: mod._hook
        sys.modules["antenv.axon_hooks"] = mod
        antenv.axon_hooks = mod
        try:
            from trn_agent_boot.trn_boot import _ntff_profile_via_ctypes

            mod._hook = _ntff_profile_via_ctypes("/opt/axon/libaxon_pjrt.so")
        except Exception:
            pass

    from concourse import bass_utils

    bass_utils.upload_artifacts = lambda tmpdir: f"local://{tmpdir}"


def host_prep(h_t, keys, W_h, W_k, v):
    bf = ml_dtypes.bfloat16
    e4 = ml_dtypes.float8_e4m3
    f32 = np.float32
    h_t = np.asarray(h_t, dtype=f32)
    keys = np.asarray(keys, dtype=f32)
    W_h = np.asarray(W_h, dtype=f32)
    W_k = np.asarray(W_k, dtype=f32)
    v = np.asarray(v, dtype=f32)

    def q8(x):
        return np.clip(x, -240.0, 240.0).astype(e4)

    # keys in three forms: bf16 natural, e4m3 transposed, e4m3 residual x256
    knat = keys.astype(bf)
    keys_T = np.ascontiguousarray(keys.transpose(0, 2, 1))  # [B, D, N]
    kt8 = q8(keys_T)
    dk8 = q8(256.0 * (kt8.astype(f32) - keys_T))

    # weights: W8 = e4m3(64*W_k); correction vectors (host fp32)
    W8s = q8(64.0 * W_k)
    W8f = W8s.astype(f32)
    w1 = (W8f / 64.0 - W_k).T @ v
    u8 = (W8f.T @ v) / 64.0
    w1_8 = q8(-C_TAYLOR * SC_SCALE * w1).reshape(D, 1)
    u8_8 = q8(-C_TAYLOR * 256.0 * u8).reshape(D, 1)
    pad8 = np.zeros((D, 14), dtype=e4)
    w8T_arr = np.concatenate([np.ascontiguousarray(W8s.T), w1_8, u8_8, pad8], axis=1)

    whT_arr = np.ascontiguousarray(W_h.T).astype(bf)
    v_s = (SC_SCALE * v).astype(bf).reshape(D, 1)
    v_f = (SC_SCALE * v).astype(f32).reshape(D, 1)
    tail_cols = np.zeros((D, 24 - B_LOC - 1), dtype=bf)
    tail_cols[0:B_LOC, 7 : 7 + B_LOC] = np.eye(B_LOC, dtype=bf)

    in_maps = []
    for c in range(NCORES):
        sl = slice(c * B_LOC, (c + 1) * B_LOC)
        htT = np.ascontiguousarray(h_t[sl].T).astype(bf)
        wsm_arr = np.concatenate([htT, v_s, tail_cols], axis=1)
        in_maps.append(
            {
                "knat_l": knat[sl],
                "kt8_l": kt8[sl],
                "dk8_l": dk8[sl],
                "w8T": w8T_arr,
                "whT": whT_arr,
                "wsm": wsm_arr,
                "vf32": v_f,
            }
        )
    return in_maps


def kernel(h_t, keys, W_h, W_k, v):
    from concourse import bass_utils

    in_maps = host_prep(h_t, keys, W_h, W_k, v)
    nc = _get_compiled()

    trace = os.environ.get("BAHDANAU_TRACE", "0") == "1"
    if trace:
        _install_prof_shim()
    res = bass_utils.run_bass_kernel_spmd(
        nc, in_maps, core_ids=list(range(NCORES)), trace=trace
    )
    if trace:
        kernel.last_exec_time_ns = res.exec_time_ns
        kernel.last_results = res

    context = np.concatenate([res.results[c]["ctx_out"] for c in range(NCORES)], axis=0)
    alpha = np.concatenate([res.results[c]["alpha_out"] for c in range(NCORES)], axis=0)
    return (context, alpha)
